# revision 1
# baseline (speedup 1.0000x reference)
"""EncDec ConvLSTM kernel for 8 Trainium2 NeuronCores.

Sharding: 8 cores = 4 (batch) x 2 (spatial row-halves). Each core computes
its 32 output rows plus a shrinking redundant halo (21-s extra rows at
recurrent step s), so no cross-core communication is needed. Row-half 1
cores receive a vertically flipped image and ky-flipped conv weights, so a
single SPMD program serves all cores.

Conv3x3 is mapped to PE matmuls over pixels (N=512 free dim, fp32r):
per 8-row tile the 4H=256 gate channels come from 2 M-tiles x 7
accumulating matmuls (1 x-im2col K=72 + 3 paired h-taps K=128 + 3 single
h-taps K=64). The kx=0/kx=2 h-taps are packed into one K=128 matmul using
a column-shifted copy of h kept in partitions 64..127.
"""

import os
import sys

import numpy as np

for _p in ("/opt/trn_rl_repo", "/root/.axon_site/_ro/trn_rl_repo"):
    if os.path.isdir(_p) and _p not in sys.path:
        sys.path.append(_p)

T = 10
F = 8
HD = 64
HS = 64
WS = 64
NCORES = 8
PW = 66  # padded grid width/height
NSTEPS = 2 * T

_CACHE = {}


def _regions():
    """Rounded compute-region row counts per recurrent step s=1..NSTEPS."""
    out = []
    for s in range(1, NSTEPS + 1):
        need = NSTEPS + 1 - s
        rows = min(HS, 32 + need)
        rows = min(HS, ((rows + 7) // 8) * 8)
        out.append(rows)
    return out


def _build_program(use_bf16=True):
    from concourse import bacc, mybir, tile

    F32 = mybir.dt.float32
    MMDT = mybir.dt.bfloat16 if use_bf16 else mybir.dt.float32r
    ACT = mybir.ActivationFunctionType

    nc = bacc.Bacc("TRN2", target_bir_lowering=False, debug=False,
                   num_devices=NCORES)

    def din(name, shape, dt=MMDT):
        return nc.dram_tensor(name, shape, dt, kind="ExternalInput").ap()

    xe_d = din("xe", [T, F, PW, PW])
    xd_d = din("xd", [T, F, PW, PW])
    w_x = {"e": din("w_ex", [72, 256]), "d": din("w_dx", [72, 256])}
    w_p = {ph: [din(f"w_{ph}p{k}", [128, 256]) for k in range(3)]
           for ph in ("e", "d")}
    # middle-column (kx=1) taps packed as two K=96 groups:
    #   A = [tap(0,1) all 64 ch; tap(1,1) ch 0:32]   vs ta = [h; h row-shift]
    #   B = [tap(1,1) ch 32:64; tap(2,1) all 64 ch]  vs tb
    w_a = {ph: din(f"w_{ph}a", [96, 256]) for ph in ("e", "d")}
    w_b = {ph: din(f"w_{ph}b", [96, 256]) for ph in ("e", "d")}
    w_op = [din(f"w_op{k}", [128, 8]) for k in range(3)]
    w_oa = din("w_oa", [96, 8])
    w_ob = din("w_ob", [96, 8])
    b_m0 = {"e": din("b_e0", [128, 1], F32), "d": din("b_d0", [128, 1], F32)}
    b_m1 = {"e": din("b_e1", [128, 1], F32), "d": din("b_d1", [128, 1], F32)}
    b_o = din("b_o", [8, 1], F32)
    zz_d = din("zz", [128, PW * PW])  # fp32r zeros for state init
    y_d = nc.dram_tensor("y", [T, F, 32, WS], F32, kind="ExternalOutput").ap()

    regions = _regions()

    with tile.TileContext(nc) as tc:
        with tc.tile_pool(name="wpool", bufs=1) as wp, \
             tc.tile_pool(name="state", bufs=1) as stp, \
             tc.tile_pool(name="x2p", bufs=2) as x2p, \
             tc.tile_pool(name="gps", bufs=6, space="PSUM") as gps, \
             tc.tile_pool(name="ops", bufs=2, space="PSUM") as ops, \
             tc.tile_pool(name="fip", bufs=3) as fip, \
             tc.tile_pool(name="ogp", bufs=3) as ogp, \
             tc.tile_pool(name="t1p", bufs=3) as t1p, \
             tc.tile_pool(name="t1lp", bufs=3) as t1lp, \
             tc.tile_pool(name="thp", bufs=3) as thp, \
             tc.tile_pool(name="yyp", bufs=2) as yyp:

            # ---- load weights / biases into SBUF ----
            def wtile(src, shape, tag, dt=MMDT):
                t_ = wp.tile(shape, dt, tag=tag)
                nc.sync.dma_start(t_[:], src[:])
                return t_

            sw_x = {ph: wtile(w_x[ph], [72, 256], f"wx{ph}")
                    for ph in ("e", "d")}
            sw_p = {ph: [wtile(w_p[ph][k], [128, 256], f"wp{ph}{k}")
                         for k in range(3)] for ph in ("e", "d")}
            sw_a = {ph: wtile(w_a[ph], [96, 256], f"wa{ph}")
                    for ph in ("e", "d")}
            sw_b = {ph: wtile(w_b[ph], [96, 256], f"wb{ph}")
                    for ph in ("e", "d")}
            sw_op = [wtile(w_op[k], [128, 8], f"wop{k}") for k in range(3)]
            sw_oa = wtile(w_oa, [96, 8], "woa")
            sw_ob = wtile(w_ob, [96, 8], "wob")
            sb_m0 = {ph: wtile(b_m0[ph], [128, 1], f"b0{ph}", F32)
                     for ph in ("e", "d")}
            sb_m1 = {ph: wtile(b_m1[ph], [128, 1], f"b1{ph}", F32)
                     for ph in ("e", "d")}
            sb_o = wtile(b_o, [8, 1], "bo", F32)

            # ---- persistent state ----
            hhA = stp.tile([128, PW * PW], MMDT, tag="hhA")
            hhB = stp.tile([128, PW * PW], MMDT, tag="hhB")
            taA = stp.tile([96, PW * PW], MMDT, tag="taA")
            taB = stp.tile([96, PW * PW], MMDT, tag="taB")
            tbA = stp.tile([96, PW * PW], MMDT, tag="tbA")
            tbB = stp.tile([96, PW * PW], MMDT, tag="tbB")
            c_t = stp.tile([64, PW * PW], F32, tag="c")
            nc.sync.dma_start(hhA[:], zz_d[:])
            nc.sync.dma_start(hhB[:], zz_d[:])
            for t_ in (taA, taB, tbA, tbB):
                nc.sync.dma_start(t_[:], zz_d[0:96])
            nc.vector.memset(c_t[:], 0.0)

            # PE clock warm-up: a dense run of small-weight matmuls keeps
            # the PE array near-100% active so HAM raises the clock to
            # 2.4GHz before the real work starts. Gate matmuls alone never
            # warm it (128-col LDWEIGHTS between every MM lowers the
            # array's duty cycle below HAM's busy threshold).
            for _ in range(64):
                wu = ops.tile([8, 512], F32, tag="pso")
                nc.tensor.matmul(wu[:], sw_op[0][:], hhA[:, 0:512],
                                 start=True, stop=True)
            def gv(t_):
                return t_[:].rearrange("p (r c) -> p r c", c=PW)

            hhAv, hhBv = gv(hhA), gv(hhB)
            taAv, taBv, tbAv, tbBv = gv(taA), gv(taB), gv(tbA), gv(tbB)
            c_v = gv(c_t)

            def emit_x2col(s):
                """Load x im2col for step s: partition (ky*3+kx)*8+ic holds
                the flat padded image shifted by ky*66+kx (contiguous)."""
                ph = "e" if s <= T else "d"
                t_idx = (s - 1) if ph == "e" else (s - 1 - T)
                x_src = xe_d if ph == "e" else xd_d
                rp = regions[s - 1]
                ln = (rp - 1) * PW + 64
                x2 = x2p.tile([72, 57 * PW], MMDT, tag="x2")
                flat = x_src[t_idx].rearrange("a r c -> a (r c)")
                for tap in range(9):
                    sh = (tap // 3) * PW + (tap % 3)
                    nc.gpsimd.dma_start(x2[tap * 8:(tap + 1) * 8, 0:ln],
                                        flat[:, sh:sh + ln])
                return x2

            def emit_outconv(s, h_view, ta_view, tb_view):
                """relu(out conv) for decoder step s, reading its h buffer."""
                t_o = s - 1 - T
                for n2 in range(4):
                    r0 = n2 * 8
                    pso = ops.tile([8, 512], F32, tag="pso")
                    for k in range(3):
                        nc.tensor.matmul(pso[:], sw_op[k][:],
                                         h_view[:, r0 + k:r0 + k + 8, 0:64],
                                         start=(k == 0), stop=False)
                    nc.tensor.matmul(pso[:], sw_oa[:],
                                     ta_view[0:96, r0:r0 + 8, 1:65],
                                     start=False, stop=False)
                    nc.tensor.matmul(pso[:], sw_ob[:],
                                     tb_view[0:96, r0 + 1:r0 + 9, 1:65],
                                     start=False, stop=True)
                    yy = yyp.tile([8, 512], F32, tag="yy")
                    nc.scalar.activation(yy[:], pso[:], ACT.Relu,
                                         bias=sb_o[:])
                    nc.gpsimd.dma_start(
                        y_d[t_o, :, r0:r0 + 8, :],
                        yy[:].rearrange("p (r c) -> p r c", c=64))

            x2_cur = emit_x2col(1)
            for s in range(1, NSTEPS + 1):
                ph = "e" if s <= T else "d"
                rp = regions[s - 1]
                ntiles = rp // 8
                if s % 2 == 0:  # read buffers written at s-1
                    h_r, ta_r, tb_r = hhAv, taAv, tbAv
                    h_w, ta_w, tb_w = hhBv, taBv, tbBv
                else:
                    h_r, ta_r, tb_r = hhBv, taBv, tbBv
                    h_w, ta_w, tb_w = hhAv, taAv, tbAv

                if s > T + 1:
                    # prev decoder step's out conv; deps long resolved
                    emit_outconv(s - 1, h_r, ta_r, tb_r)
                x2v = x2_cur[:].rearrange("p (r c) -> p r c", c=PW)
                if s < NSTEPS:
                    x2_next = emit_x2col(s + 1)  # prefetch on gpsimd queue

                if 1 < s <= T:
                    # re-warm burst: dense small-weight matmuls (8-col
                    # LDWEIGHTS) keep/restore the PE clock; the gate matmul
                    # pattern alone (128-col LDWEIGHTS per MM) falls under
                    # HAM's busy threshold. Decoder steps get this for free
                    # from the out-conv blocks. Reads x2 (read-only here).
                    for _ in range(10):
                        wu = ops.tile([8, 512], F32, tag="pso")
                        nc.tensor.matmul(wu[:], sw_oa[0:64, :],
                                         x2v[0:64, 0:8, 0:64],
                                         start=True, stop=True)

                for n in range(ntiles):
                    r0 = n * 8
                    ps0 = gps.tile([128, 512], F32, tag="ps")
                    ps1 = gps.tile([128, 512], F32, tag="ps")
                    for m, ps in ((0, ps0), (1, ps1)):
                        ms = slice(m * 128, (m + 1) * 128)
                        nc.tensor.matmul(ps[:], sw_x[ph][:, ms],
                                         x2v[0:72, r0:r0 + 8, 0:64],
                                         start=True, stop=False)
                        for k in range(3):
                            nc.tensor.matmul(
                                ps[:], sw_p[ph][k][:, ms],
                                h_r[:, r0 + k:r0 + k + 8, 0:64],
                                start=False, stop=False)
                        nc.tensor.matmul(ps[:], sw_a[ph][:, ms],
                                         ta_r[0:96, r0:r0 + 8, 1:65],
                                         start=False, stop=False)
                        nc.tensor.matmul(ps[:], sw_b[ph][:, ms],
                                         tb_r[0:96, r0 + 1:r0 + 9, 1:65],
                                         start=False, stop=True)

                    # epilogue: M0=[f;i] M1=[o;g]
                    fi = fip.tile([128, 512], F32, tag="fi")
                    og = ogp.tile([128, 512], F32, tag="og")
                    nc.scalar.activation(fi[:], ps0[:], ACT.Sigmoid,
                                         bias=sb_m0[ph][:])
                    nc.scalar.activation(og[0:64], ps1[0:64], ACT.Sigmoid,
                                         bias=sb_m1[ph][0:64])
                    nc.scalar.activation(og[64:128], ps1[64:128], ACT.Tanh,
                                         bias=sb_m1[ph][64:128])
                    # t1 = sigmoid(i) * tanh(g) on partitions 64..127
                    t1 = t1p.tile([128, 512], F32, tag="t1")
                    nc.vector.tensor_mul(t1[64:128], fi[64:128], og[64:128])
                    # cross-partition move 64..127 -> 0..63
                    t1l = t1lp.tile([64, 512], F32, tag="t1l")
                    nc.sync.dma_start(t1l[:], t1[64:128])
                    t1lv = t1l[:].rearrange("p (r c) -> p r c", c=64)
                    cs = c_v[0:64, r0 + 1:r0 + 9, 1:65]
                    nc.vector.tensor_mul(cs, cs, fi[0:64].rearrange(
                        "p (r c) -> p r c", c=64))
                    nc.vector.tensor_add(cs, cs, t1lv)
                    th = thp.tile([64, 512], F32, tag="th")
                    thv = th[:].rearrange("p (r c) -> p r c", c=64)
                    nc.scalar.activation(thv, cs, ACT.Tanh)
                    # h = tanh(c) * sigmoid(o) -> base half of write buffer
                    nc.vector.tensor_mul(
                        h_w[0:64, r0 + 1:r0 + 9, 1:65], thv,
                        og[0:64].rearrange("p (r c) -> p r c", c=64))
                    # shifted copy (cols +2) into partitions 64..127
                    nc.sync.dma_start(
                        h_w[64:128, r0 + 1:r0 + 9, 0:64],
                        h_w[0:64, r0 + 1:r0 + 9, 2:66])
                    # fan out h into the K=96 middle-column group tiles:
                    # ta = [h ch0:64 (row 0-base); h ch0:32 shifted one row]
                    # tb = [h ch32:64 (row 1-base); h ch0:64 shifted one row]
                    nc.sync.dma_start(ta_w[0:64, r0 + 1:r0 + 9, 1:65],
                                      h_w[0:64, r0 + 1:r0 + 9, 1:65])
                    nc.sync.dma_start(ta_w[64:96, r0:r0 + 8, 1:65],
                                      h_w[0:32, r0 + 1:r0 + 9, 1:65])
                    nc.sync.dma_start(tb_w[0:32, r0 + 1:r0 + 9, 1:65],
                                      h_w[32:64, r0 + 1:r0 + 9, 1:65])
                    nc.sync.dma_start(tb_w[32:96, r0:r0 + 8, 1:65],
                                      h_w[0:64, r0 + 1:r0 + 9, 1:65])

                if s < NSTEPS:
                    x2_cur = x2_next

            # out conv for the final decoder step (NSTEPS is even -> B bufs)
            emit_outconv(NSTEPS, hhBv, taBv, tbBv)

    nc.compile()
    return nc


def _prep_core_inputs(core, enc_in, dec_in, enc_W, enc_b, dec_W, dec_b,
                      out_W, out_b, use_bf16=True):
    import ml_dtypes
    mm_np = ml_dtypes.bfloat16 if use_bf16 else np.float32
    b, half = core // 2, core % 2
    # gate permutation: [f, i, o, g]
    perm = np.concatenate([np.arange(0, 128), np.arange(192, 256),
                           np.arange(128, 192)])

    def prep_x(x):
        x = x[b]  # [T, F, 64, 64]
        if half:
            x = x[:, :, ::-1, :]
        xp = np.zeros((T, F, PW, PW), np.float32)
        xp[:, :, 1:65, 1:65] = x
        return np.ascontiguousarray(xp)

    def prep_gateW(W, bias):
        Wf = W[:, :, ::-1, :] if half else W
        Wp = np.ascontiguousarray(Wf[perm])  # [256, 72, 3, 3]
        bp = bias[perm].astype(np.float32)
        # x part: rows (ky*3+kx)*8+ic
        lx = Wp[:, :F].transpose(2, 3, 1, 0).reshape(72, 256)
        lp = [np.concatenate([Wp[:, F:, k, 0].T, Wp[:, F:, k, 2].T], axis=0)
              for k in range(3)]  # [128, 256]
        la = np.concatenate([Wp[:, F:, 0, 1].T, Wp[:, F:F + 32, 1, 1].T],
                            axis=0)  # [96, 256]
        lb = np.concatenate([Wp[:, F + 32:, 1, 1].T, Wp[:, F:, 2, 1].T],
                            axis=0)  # [96, 256]
        return (np.ascontiguousarray(lx),
                [np.ascontiguousarray(a) for a in lp],
                np.ascontiguousarray(la), np.ascontiguousarray(lb),
                np.ascontiguousarray(bp[0:128].reshape(128, 1)),
                np.ascontiguousarray(bp[128:256].reshape(128, 1)))

    ex, ep, ea, eb, eb0, eb1 = prep_gateW(enc_W, enc_b)
    dx, dp, da, db, db0, db1 = prep_gateW(dec_W, dec_b)
    oWf = out_W[:, :, ::-1, :] if half else out_W
    op = [np.ascontiguousarray(np.concatenate(
        [oWf[:, :, k, 0].T, oWf[:, :, k, 2].T], axis=0).astype(np.float32))
        for k in range(3)]
    oa = np.ascontiguousarray(np.concatenate(
        [oWf[:, :, 0, 1].T, oWf[:, 0:32, 1, 1].T], axis=0))
    ob = np.ascontiguousarray(np.concatenate(
        [oWf[:, 32:64, 1, 1].T, oWf[:, :, 2, 1].T], axis=0))

    m = {"xe": prep_x(enc_in), "xd": prep_x(dec_in),
         "w_ex": ex, "w_dx": dx,
         "w_ea": ea, "w_eb": eb, "w_da": da, "w_db": db,
         "w_oa": oa, "w_ob": ob,
         "b_e0": eb0, "b_e1": eb1, "b_d0": db0, "b_d1": db1,
         "b_o": np.ascontiguousarray(out_b.reshape(8, 1).astype(np.float32)),
         "zz": np.zeros((128, PW * PW), np.float32)}
    for k in range(3):
        m[f"w_ep{k}"] = ep[k]
        m[f"w_dp{k}"] = dp[k]
        m[f"w_op{k}"] = op[k]
    f32_keys = {"b_e0", "b_e1", "b_d0", "b_d1", "b_o"}
    return {k: np.ascontiguousarray(np.asarray(
        v, np.float32 if k in f32_keys else mm_np)) for k, v in m.items()}


def _install_trace_hook():
    """Shim antenv.axon_hooks for NTFF profiling (dev only)."""
    import contextlib
    import ctypes
    import types

    so = "/opt/axon/libaxon_pjrt.so"
    if "antenv.axon_hooks" in sys.modules or not os.path.exists(so):
        return
    lib = ctypes.CDLL(so)
    if not hasattr(lib, "axon_start_nrt_profile"):
        return
    lib.axon_start_nrt_profile.argtypes = [ctypes.POINTER(ctypes.c_int64),
                                           ctypes.c_size_t]
    lib.axon_start_nrt_profile.restype = ctypes.c_int64
    lib.axon_stop_nrt_profile.argtypes = [ctypes.c_char_p]
    lib.axon_stop_nrt_profile.restype = ctypes.c_int64

    def _mk():
        @contextlib.contextmanager
        def _hook(output_dir, device_ids):
            import jax
            jax.devices()
            if device_ids:
                ids = (ctypes.c_int64 * len(device_ids))(*device_ids)
                rc = lib.axon_start_nrt_profile(ids, len(device_ids))
            else:
                rc = lib.axon_start_nrt_profile(None, 0)
            if rc != 0:
                raise RuntimeError(f"axon_start_nrt_profile rc={rc}")
            try:
                yield
            finally:
                lib.axon_stop_nrt_profile(str(output_dir).encode())
        return _hook

    mod = types.ModuleType("antenv.axon_hooks")
    mod.get_axon_ntff_profile_hook = _mk
    sys.modules["antenv.axon_hooks"] = mod


def kernel(enc_in, dec_in, enc_W, enc_b, dec_W, dec_b, out_W, out_b):
    from concourse.bass_utils import run_bass_kernel_spmd

    trace = os.environ.get("KERNEL_TRACE", "") == "1"
    if trace:
        _install_trace_hook()

    use_bf16 = os.environ.get("KERNEL_DTYPE", "bf16") != "f32r"
    if "nc" not in _CACHE:
        _CACHE["nc"] = _build_program(use_bf16)
    nc = _CACHE["nc"]

    args = (np.asarray(enc_in, np.float32), np.asarray(dec_in, np.float32),
            np.asarray(enc_W, np.float32), np.asarray(enc_b, np.float32),
            np.asarray(dec_W, np.float32), np.asarray(dec_b, np.float32),
            np.asarray(out_W, np.float32), np.asarray(out_b, np.float32))
    in_maps = [_prep_core_inputs(c, *args, use_bf16=use_bf16)
               for c in range(NCORES)]

    res = run_bass_kernel_spmd(nc, in_maps, list(range(NCORES)), trace=trace)
    if trace:
        _CACHE["exec_time_ns"] = res.exec_time_ns

    B = enc_in.shape[0]
    out = np.empty((B, T, F, HS, WS), np.float32)
    for c in range(NCORES):
        b, half = c // 2, c % 2
        yc = res.results[c]["y"]  # [T, F, 32, 64]
        if half:
            out[b, :, :, 32:64, :] = yc[:, :, ::-1, :]
        else:
            out[b, :, :, 0:32, :] = yc
    return out



# revision 10
# speedup vs baseline: 1.3889x; 1.3889x over previous
"""EncDec ConvLSTM kernel for 8 Trainium2 NeuronCores.

Sharding: 8 cores = 4 (batch) x 2 (spatial row-halves). Each core computes
its 32 output rows plus a shrinking redundant halo (21-s extra rows at
recurrent step s), so no cross-core communication is needed. Row-half 1
cores receive a vertically flipped image and ky-flipped conv weights, so a
single SPMD program serves all cores.

Conv3x3 is mapped to PE matmuls over pixels (N=512 free dim, bf16).
State tile R[128, grid]: partitions 0:64 hold h, partitions 64:128 hold h
col-shifted by +2 (one SBUF->SBUF DMA per tile, off the critical path).
Per 8-row tile and M-tile: 1 x-im2col MM (K=72), 3 paired-kx MMs (K=128,
reading [h | h+2col] at row offsets 0/1/2), and 3 middle-column taps as
K=64 row-strip MMs reading the same tile: ky=0,2 from the lower half at
col offset +1, ky=1 from the upper half at col offset -1 (the col-shifted
copy re-read one col left IS the middle column). No ta/tb packed copies.
The sig(i)*tanh(g) product is written cross-partition (in@64:128 ->
out@0:63) directly by the DVE, eliminating the old t1l DMA.
"""

import os
import sys

import numpy as np

for _p in ("/opt/trn_rl_repo", "/root/.axon_site/_ro/trn_rl_repo"):
    if os.path.isdir(_p) and _p not in sys.path:
        sys.path.append(_p)

T = 10
F = 8
HD = 64
HS = 64
WS = 64
NCORES = 8
PW = 66   # padded grid width/height
LEAD = 66  # one extra leading pad row in the R state tile
RSZ = LEAD + PW * PW + 2  # flat elems per partition in R
NSTEPS = 2 * T

_CACHE = {}


def _regions():
    """Rounded compute-region row counts per recurrent step s=1..NSTEPS."""
    out = []
    for s in range(1, NSTEPS + 1):
        need = NSTEPS + 1 - s
        rows = min(HS, 32 + need)
        rows = min(HS, ((rows + 7) // 8) * 8)
        out.append(rows)
    return out


def _build_program(use_bf16=True):
    from concourse import bacc, mybir, tile

    F32 = mybir.dt.float32
    MMDT = mybir.dt.bfloat16 if use_bf16 else mybir.dt.float32r
    ACT = mybir.ActivationFunctionType

    nc = bacc.Bacc("TRN2", target_bir_lowering=False, debug=False,
                   num_devices=NCORES)

    def din(name, shape, dt=MMDT):
        return nc.dram_tensor(name, shape, dt, kind="ExternalInput").ap()

    xe_d = din("xe", [T, F, PW, PW])
    xd_d = din("xd", [T, F, PW, PW])
    w_x = {"e": din("w_ex", [72, 256]), "d": din("w_dx", [72, 256])}
    w_p = {ph: [din(f"w_{ph}p{k}", [128, 256]) for k in range(3)]
           for ph in ("e", "d")}
    # middle-column (kx=1) taps: mA = [ky0 (strip 0); ky1 (strip 64)],
    # mB = [ky2] (strip 0, K=64); m1lo = ky1 at base 0 (non-strip variant)
    w_ma = {ph: din(f"w_{ph}ma", [128, 256]) for ph in ("e", "d")}
    w_mb = {ph: din(f"w_{ph}mb", [64, 256]) for ph in ("e", "d")}
    w_m1lo = {ph: din(f"w_{ph}m1lo", [64, 256]) for ph in ("e", "d")}
    w_op = [din(f"w_op{k}", [128, 8]) for k in range(3)]
    w_oma = din("w_oma", [128, 8])
    w_omb = din("w_omb", [64, 8])
    w_om1lo = din("w_om1lo", [64, 8])
    use_strip = os.environ.get("KERNEL_STRIP", "1") == "1"
    use_xbase = os.environ.get("KERNEL_XBASE", "1") == "1"
    b_m0 = {"e": din("b_e0", [128, 1], F32), "d": din("b_d0", [128, 1], F32)}
    b_m1 = {"e": din("b_e1", [128, 1], F32), "d": din("b_d1", [128, 1], F32)}
    b_o = din("b_o", [8, 1], F32)
    y_d = nc.dram_tensor("y", [T, F, 32, WS], F32, kind="ExternalOutput").ap()

    regions = _regions()

    with tile.TileContext(nc) as tc:
        with tc.tile_pool(name="wpool", bufs=1) as wp, \
             tc.tile_pool(name="state", bufs=1) as stp, \
             tc.tile_pool(name="x2p", bufs=2) as x2p, \
             tc.tile_pool(name="gps", bufs=6, space="PSUM") as gps, \
             tc.tile_pool(name="ops", bufs=2, space="PSUM") as ops, \
             tc.tile_pool(name="fip", bufs=3) as fip, \
             tc.tile_pool(name="ogp", bufs=3) as ogp, \
             tc.tile_pool(name="t1p", bufs=3) as t1p, \
             tc.tile_pool(name="thp", bufs=3) as thp, \
             tc.tile_pool(name="yyp", bufs=2) as yyp:

            # ---- load weights / biases into SBUF ----
            def wtile(src, shape, tag, dt=MMDT):
                t_ = wp.tile(shape, dt, tag=tag)
                nc.sync.dma_start(t_[:], src[:])
                return t_

            sw_x = {ph: wtile(w_x[ph], [72, 256], f"wx{ph}")
                    for ph in ("e", "d")}
            sw_p = {ph: [wtile(w_p[ph][k], [128, 256], f"wp{ph}{k}")
                         for k in range(3)] for ph in ("e", "d")}
            sw_ma = {ph: wtile(w_ma[ph], [128, 256], f"wma{ph}")
                     for ph in ("e", "d")}
            sw_mb = {ph: wtile(w_mb[ph], [64, 256], f"wmb{ph}")
                     for ph in ("e", "d")}
            sw_m1lo = {ph: wtile(w_m1lo[ph], [64, 256], f"wm1lo{ph}")
                       for ph in ("e", "d")}
            sw_op = [wtile(w_op[k], [128, 8], f"wop{k}") for k in range(3)]
            sw_oma = wtile(w_oma, [128, 8], "woma")
            sw_omb = wtile(w_omb, [64, 8], "womb")
            sw_om1lo = wtile(w_om1lo, [64, 8], "wom1lo")
            sb_m0 = {ph: wtile(b_m0[ph], [128, 1], f"b0{ph}", F32)
                     for ph in ("e", "d")}
            sb_m1 = {ph: wtile(b_m1[ph], [128, 1], f"b1{ph}", F32)
                     for ph in ("e", "d")}
            sb_o = wtile(b_o, [8, 1], "bo", F32)

            # ---- persistent state ----
            # R: [h (parts 0:64) | h col-shifted +2 (parts 64:128)]
            rrA = stp.tile([128, RSZ], MMDT, tag="rrA")
            rrB = stp.tile([128, RSZ], MMDT, tag="rrB")
            c_t = stp.tile([64, HS * WS], F32, tag="c")
            nc.vector.memset(rrA[:], 0.0)
            nc.vector.memset(rrB[:], 0.0)
            nc.vector.memset(c_t[:], 0.0)

            # PE clock warm-up: ~3.4us of sustained matmul activity raises
            # the HAM clock gate to 8/8 before the real work starts.
            for _ in range(24):
                wu = ops.tile([8, 512], F32, tag="pso")
                nc.tensor.matmul(wu[:], sw_op[0][:], rrA[0:128, 0:512],
                                 start=True, stop=True)

            def gview(t_, p0, p1, flat_off, nr=8):
                """[p1-p0, nr, 64] view of grid tile at flat elem offset."""
                v = t_[p0:p1, flat_off:flat_off + nr * PW]
                v = v.rearrange("p (r c) -> p r c", c=PW)
                return v[:, 0:nr, 0:64]

            def emit_x2col(s):
                """Load x im2col for step s: partition (ky*3+kx)*8+ic holds
                the flat padded image shifted by ky*66+kx (contiguous)."""
                ph = "e" if s <= T else "d"
                t_idx = (s - 1) if ph == "e" else (s - 1 - T)
                x_src = xe_d if ph == "e" else xd_d
                rp = regions[s - 1]
                ln = (rp - 1) * PW + 64
                x2 = x2p.tile([72, 57 * PW], MMDT, tag="x2")
                flat = x_src[t_idx].rearrange("a r c -> a (r c)")
                for tap in range(9):
                    sh = (tap // 3) * PW + (tap % 3)
                    nc.gpsimd.dma_start(x2[tap * 8:(tap + 1) * 8, 0:ln],
                                        flat[:, sh:sh + ln])
                return x2

            def gate_mms(ps, wx, wp3, wma, wmb, wm1lo, ms, x2v, R, r0):
                """Accumulate all 4H-gate conv taps for one M-tile."""
                nc.tensor.matmul(ps[:], wx[:, ms],
                                 x2v[0:72, r0:r0 + 8, 0:64],
                                 start=True, stop=False)
                for k in range(3):
                    nc.tensor.matmul(
                        ps[:], wp3[k][:, ms],
                        gview(R, 0, 128, LEAD + (r0 + k) * PW),
                        start=False, stop=False)
                # middle column: ky=0 lower(+1), ky=1 upper(-1) -- these two
                # run concurrently on row strips (0,0)/(64,0) -- ky=2 lower
                nc.tensor.matmul(ps[:], wma[0:64, ms],
                                 gview(R, 0, 64, LEAD + r0 * PW + 1),
                                 start=False, stop=False)
                if use_strip:
                    nc.tensor.matmul(
                        ps[:], wma[64:128, ms],
                        gview(R, 64, 128, LEAD + (r0 + 1) * PW - 1),
                        start=False, stop=False)
                else:
                    nc.tensor.matmul(
                        ps[:], wm1lo[:, ms],
                        gview(R, 0, 64, LEAD + (r0 + 1) * PW + 1),
                        start=False, stop=False)
                nc.tensor.matmul(ps[:], wmb[:, ms],
                                 gview(R, 0, 64, LEAD + (r0 + 2) * PW + 1),
                                 start=False, stop=True)

            def emit_outconv(s, R):
                """relu(out conv) for decoder step s, reading its R buffer."""
                t_o = s - 1 - T
                for n2 in range(4):
                    r0 = n2 * 8
                    pso = ops.tile([8, 512], F32, tag="pso")
                    for k in range(3):
                        nc.tensor.matmul(pso[:], sw_op[k][:],
                                         gview(R, 0, 128, LEAD + (r0 + k) * PW),
                                         start=(k == 0), stop=False)
                    nc.tensor.matmul(pso[:], sw_oma[0:64, :],
                                     gview(R, 0, 64, LEAD + r0 * PW + 1),
                                     start=False, stop=False)
                    if use_strip:
                        nc.tensor.matmul(pso[:], sw_oma[64:128, :],
                                         gview(R, 64, 128,
                                               LEAD + (r0 + 1) * PW - 1),
                                         start=False, stop=False)
                    else:
                        nc.tensor.matmul(pso[:], sw_om1lo[:, :],
                                         gview(R, 0, 64,
                                               LEAD + (r0 + 1) * PW + 1),
                                         start=False, stop=False)
                    nc.tensor.matmul(pso[:], sw_omb[:, :],
                                     gview(R, 0, 64, LEAD + (r0 + 2) * PW + 1),
                                     start=False, stop=True)
                    yy = yyp.tile([8, 512], F32, tag="yy")
                    nc.scalar.activation(yy[:], pso[:], ACT.Relu,
                                         bias=sb_o[:])
                    nc.gpsimd.dma_start(
                        y_d[t_o, :, r0:r0 + 8, :],
                        yy[:].rearrange("p (r c) -> p r c", c=64))

            x2_cur = emit_x2col(1)
            for s in range(1, NSTEPS + 1):
                ph = "e" if s <= T else "d"
                rp = regions[s - 1]
                ntiles = rp // 8
                if s % 2 == 0:  # read buffers written at s-1
                    R_r, R_w = rrA, rrB
                else:
                    R_r, R_w = rrB, rrA

                if s > T + 1:
                    # prev decoder step's out conv; deps long resolved
                    emit_outconv(s - 1, R_r)
                x2v = x2_cur[:].rearrange("p (r c) -> p r c", c=PW)
                if s < NSTEPS:
                    x2_next = emit_x2col(s + 1)  # prefetch on gpsimd queue

                for n in range(ntiles):
                    r0 = n * 8
                    ps0 = gps.tile([128, 512], F32, tag="ps")
                    ps1 = gps.tile([128, 512], F32, tag="ps")
                    gate_mms(ps0, sw_x[ph], sw_p[ph], sw_ma[ph], sw_mb[ph],
                             sw_m1lo[ph], slice(0, 128), x2v, R_r, r0)
                    gate_mms(ps1, sw_x[ph], sw_p[ph], sw_ma[ph], sw_mb[ph],
                             sw_m1lo[ph], slice(128, 256), x2v, R_r, r0)

                    # epilogue: M0=[f;i] M1=[o;g]
                    fi = fip.tile([128, 512], F32, tag="fi")
                    og = ogp.tile([128, 512], F32, tag="og")
                    nc.scalar.activation(fi[:], ps0[:], ACT.Sigmoid,
                                         bias=sb_m0[ph][:])
                    nc.scalar.activation(og[0:64], ps1[0:64], ACT.Sigmoid,
                                         bias=sb_m1[ph][0:64])
                    nc.scalar.activation(og[64:128], ps1[64:128], ACT.Tanh,
                                         bias=sb_m1[ph][64:128])
                    # t1 = sigmoid(i)*tanh(g): cross-partition DVE write
                    # (reads from parts 64:128, writes parts 0:64)
                    if use_xbase:
                        t1 = t1p.tile([64, 512], F32, tag="t1")
                        nc.vector.tensor_mul(t1[:], fi[64:128], og[64:128])
                    else:
                        t1u = t1p.tile([128, 512], F32, tag="t1u")
                        nc.vector.tensor_mul(t1u[64:128], fi[64:128],
                                             og[64:128])
                        t1 = t1p.tile([64, 512], F32, tag="t1")
                        nc.sync.dma_start(t1[:], t1u[64:128])
                    cs = c_t[:, r0 * 64:r0 * 64 + 512]
                    nc.vector.tensor_mul(cs, cs, fi[0:64])
                    nc.vector.tensor_add(cs, cs, t1[:])
                    th = thp.tile([64, 512], MMDT, tag="th")
                    nc.scalar.activation(th[:], cs, ACT.Tanh)
                    # h = tanh(c) * sigmoid(o) -> lower half of write buffer
                    thv = th[:].rearrange("p (r c) -> p r c", c=64)
                    nc.vector.tensor_mul(
                        gview(R_w, 0, 64, LEAD + (r0 + 1) * PW + 1), thv,
                        og[0:64].rearrange("p (r c) -> p r c", c=64))
                    # col-shifted copy (+2) into partitions 64:128
                    nc.sync.dma_start(
                        gview(R_w, 64, 128, LEAD + (r0 + 1) * PW - 1),
                        gview(R_w, 0, 64, LEAD + (r0 + 1) * PW + 1))

                if s < NSTEPS:
                    x2_cur = x2_next

            # out conv for the final decoder step (NSTEPS even -> B buffer)
            emit_outconv(NSTEPS, rrB)

    nc.compile()
    return nc


def _prep_core_inputs(core, enc_in, dec_in, enc_W, enc_b, dec_W, dec_b,
                      out_W, out_b, use_bf16=True):
    import ml_dtypes
    mm_np = ml_dtypes.bfloat16 if use_bf16 else np.float32
    b, half = core // 2, core % 2
    # gate permutation: [f, i, o, g]
    perm = np.concatenate([np.arange(0, 128), np.arange(192, 256),
                           np.arange(128, 192)])

    def prep_x(x):
        x = x[b]  # [T, F, 64, 64]
        if half:
            x = x[:, :, ::-1, :]
        xp = np.zeros((T, F, PW, PW), np.float32)
        xp[:, :, 1:65, 1:65] = x
        return np.ascontiguousarray(xp)

    def prep_gateW(W, bias):
        Wf = W[:, :, ::-1, :] if half else W
        Wp = np.ascontiguousarray(Wf[perm])  # [256, 72, 3, 3]
        bp = bias[perm].astype(np.float32)
        # x part: rows (ky*3+kx)*8+ic
        lx = Wp[:, :F].transpose(2, 3, 1, 0).reshape(72, 256)
        lp = [np.concatenate([Wp[:, F:, k, 0].T, Wp[:, F:, k, 2].T], axis=0)
              for k in range(3)]  # [128, 256]
        lma = np.concatenate([Wp[:, F:, 0, 1].T, Wp[:, F:, 1, 1].T],
                             axis=0)  # [128, 256]
        lmb = np.ascontiguousarray(Wp[:, F:, 2, 1].T)  # [64, 256]
        lm1 = np.ascontiguousarray(Wp[:, F:, 1, 1].T)  # [64, 256]
        return (np.ascontiguousarray(lx),
                [np.ascontiguousarray(a) for a in lp],
                np.ascontiguousarray(lma), lmb, lm1,
                np.ascontiguousarray(bp[0:128].reshape(128, 1)),
                np.ascontiguousarray(bp[128:256].reshape(128, 1)))

    ex, ep, ema, emb, em1, eb0, eb1 = prep_gateW(enc_W, enc_b)
    dx, dp, dma_, dmb, dm1, db0, db1 = prep_gateW(dec_W, dec_b)
    oWf = out_W[:, :, ::-1, :] if half else out_W
    op = [np.ascontiguousarray(np.concatenate(
        [oWf[:, :, k, 0].T, oWf[:, :, k, 2].T], axis=0).astype(np.float32))
        for k in range(3)]
    oma = np.ascontiguousarray(np.concatenate(
        [oWf[:, :, 0, 1].T, oWf[:, :, 1, 1].T], axis=0))
    omb = np.ascontiguousarray(oWf[:, :, 2, 1].T)
    om1 = np.ascontiguousarray(oWf[:, :, 1, 1].T)

    m = {"xe": prep_x(enc_in), "xd": prep_x(dec_in),
         "w_ex": ex, "w_dx": dx,
         "w_ema": ema, "w_emb": emb, "w_dma": dma_, "w_dmb": dmb,
         "w_em1lo": em1, "w_dm1lo": dm1,
         "w_oma": oma, "w_omb": omb, "w_om1lo": om1,
         "b_e0": eb0, "b_e1": eb1, "b_d0": db0, "b_d1": db1,
         "b_o": np.ascontiguousarray(out_b.reshape(8, 1).astype(np.float32))}
    for k in range(3):
        m[f"w_ep{k}"] = ep[k]
        m[f"w_dp{k}"] = dp[k]
        m[f"w_op{k}"] = op[k]
    f32_keys = {"b_e0", "b_e1", "b_d0", "b_d1", "b_o"}
    return {k: np.ascontiguousarray(np.asarray(
        v, np.float32 if k in f32_keys else mm_np)) for k, v in m.items()}


def _install_trace_hook():
    """Shim antenv.axon_hooks for NTFF profiling (dev only)."""
    import contextlib
    import ctypes
    import types

    so = "/opt/axon/libaxon_pjrt.so"
    if "antenv.axon_hooks" in sys.modules or not os.path.exists(so):
        return
    lib = ctypes.CDLL(so)
    if not hasattr(lib, "axon_start_nrt_profile"):
        return
    lib.axon_start_nrt_profile.argtypes = [ctypes.POINTER(ctypes.c_int64),
                                           ctypes.c_size_t]
    lib.axon_start_nrt_profile.restype = ctypes.c_int64
    lib.axon_stop_nrt_profile.argtypes = [ctypes.c_char_p]
    lib.axon_stop_nrt_profile.restype = ctypes.c_int64

    def _mk():
        @contextlib.contextmanager
        def _hook(output_dir, device_ids):
            import jax
            jax.devices()
            if device_ids:
                ids = (ctypes.c_int64 * len(device_ids))(*device_ids)
                rc = lib.axon_start_nrt_profile(ids, len(device_ids))
            else:
                rc = lib.axon_start_nrt_profile(None, 0)
            if rc != 0:
                raise RuntimeError(f"axon_start_nrt_profile rc={rc}")
            try:
                yield
            finally:
                lib.axon_stop_nrt_profile(str(output_dir).encode())
        return _hook

    mod = types.ModuleType("antenv.axon_hooks")
    mod.get_axon_ntff_profile_hook = _mk
    sys.modules["antenv.axon_hooks"] = mod


def kernel(enc_in, dec_in, enc_W, enc_b, dec_W, dec_b, out_W, out_b):
    from concourse.bass_utils import run_bass_kernel_spmd

    trace = os.environ.get("KERNEL_TRACE", "") == "1"
    if trace:
        _install_trace_hook()

    use_bf16 = os.environ.get("KERNEL_DTYPE", "bf16") != "f32r"
    if "nc" not in _CACHE:
        _CACHE["nc"] = _build_program(use_bf16)
    nc = _CACHE["nc"]

    args = (np.asarray(enc_in, np.float32), np.asarray(dec_in, np.float32),
            np.asarray(enc_W, np.float32), np.asarray(enc_b, np.float32),
            np.asarray(dec_W, np.float32), np.asarray(dec_b, np.float32),
            np.asarray(out_W, np.float32), np.asarray(out_b, np.float32))
    in_maps = [_prep_core_inputs(c, *args, use_bf16=use_bf16)
               for c in range(NCORES)]

    res = run_bass_kernel_spmd(nc, in_maps, list(range(NCORES)), trace=trace)
    if trace:
        _CACHE["exec_time_ns"] = res.exec_time_ns

    B = enc_in.shape[0]
    out = np.empty((B, T, F, HS, WS), np.float32)
    for c in range(NCORES):
        b, half = c // 2, c % 2
        yc = res.results[c]["y"]  # [T, F, 32, 64]
        if half:
            out[b, :, :, 32:64, :] = yc[:, :, ::-1, :]
        else:
            out[b, :, :, 0:32, :] = yc
    return out
